# revision 32
# baseline (speedup 1.0000x reference)
"""Trainium2 Bass kernel for the recurrent Dense module.

Math (see reference):
    pre       = inputs @ W.T + b_vec            # [B, OUT]
    out       = pre + Aux[:,0] * state_vec      # [B, OUT]
    new_state = Aux[:,1] * state_vec + mean(pre, axis=0)   # [OUT]

Distribution: the batch (8192) is sharded over the 8 NeuronCores (1024 rows
each); W / b_vec / Aux / state_vec are replicated.  Each core computes its
[1024, OUT] slice of `out`; the host concatenates the slices.  `new_state`
depends on the batch only through mean(pre, 0) which by linearity equals
(mean_b inputs) @ W.T + b_vec, a [IN]x[IN,OUT] matvec the host computes
directly in float64.

On-chip layout per core: the matmul contracts over IN, so both operands are
staged with IN on the SBUF partition axis: lhsT = x_c.T (IN x 1024, the
stationary operand) and rhs = W.T (IN x OUT, the moving operand); both
transposes are done host-side during sharding.  PSUM tiles are
[128 batch, 512 out] fp32, accumulated over 8 k-tiles.  The affine shift
b_vec + Aux[:,0]*state_vec is folded in by the PSUM->SBUF eviction add.

Precision mode (BASS_DENSE_MODE env var, default fp16), measured on HW:
  fp16    1 matmul pass, operands rounded to fp16       (2.5e-4 rel err, ~47us)
  bf16    1 matmul pass, operands rounded to bf16       (2.0e-3 rel err, ~47us)
  bf16x3  3 passes hi/hi hi/lo lo/hi of a bf16 split    (3.8e-6 rel err, ~97us)
  fp32    native fp32 matmuls (4 cycles/row)            (bit-exact,     ~130us)
"""

import os

import ml_dtypes
import numpy as np

B, IN, OUT = 8192, 1024, 1024
N_CORES = 8
BS = B // N_CORES  # batch rows per core
P = 128  # SBUF partitions
NFREE = 512  # psum tile free dim (one bank of fp32)
KT = IN // P  # contraction tiles
MT = BS // P  # batch tiles per core
NT = OUT // NFREE  # out-feature tiles

MODE = os.environ.get("BASS_DENSE_MODE", "fp16")

_cache: dict = {}


def _build(mode, with_shift=True):
    import concourse.bacc as bacc
    import concourse.mybir as mybir
    import concourse.tile as tile

    nc = bacc.Bacc(enable_partition_id=False)
    f32 = mybir.dt.float32
    if mode in ("bf16", "bf16x3"):
        dt_in = mybir.dt.bfloat16
    elif mode in ("fp16", "fp16x3"):
        dt_in = mybir.dt.float16
    elif mode == "fp32":
        dt_in = f32
    else:
        raise ValueError(mode)
    two_level = mode in ("bf16x3", "fp16x3")

    # x and W (and their low halves in x3 modes) are packed side by side in
    # one [IN, width] tensor so each k-tile is a single DMA.
    width = (BS + OUT) * (2 if two_level else 1)
    xw_d = nc.dram_tensor("xw", [IN, width], dt_in, kind="ExternalInput")
    if with_shift:
        shift_d = nc.dram_tensor("shift", [P, OUT], f32, kind="ExternalInput")
    out_d = nc.dram_tensor("out", [BS, OUT], f32, kind="ExternalOutput")

    # PE warm-up, emitted in the main block so it runs during the framework
    # preamble, before the Tile block is even entered.  The HAM clock gate
    # needs ~3.4us of sustained PE activity before it lifts the PE from 1.2
    # to 2.4 GHz; these N=1 matmuls on the framework's const tiles provide it
    # so the real matmuls run at full clock from the first one.  The scratch
    # PSUM bank is freed again before the Tile pools allocate, and every real
    # accumulation group opens with start=True, which discards whatever the
    # warm-up wrote.
    NPRE = int(os.environ.get("BASS_DENSE_PREWARM", "0"))
    if NPRE:
        # Full-width (N=512) matmuls: narrow ones don't register enough PE
        # activity to trip the gate.  The scratch SBUF operand is read
        # uninitialized; its product is discarded, so the values are
        # irrelevant.
        warm_sb = nc.alloc_sbuf_tensor("warm_sb", [P, P + NFREE], mybir.dt.bfloat16)
        with nc.psum_tensor("warm_ps", [P, NFREE], f32) as wps:
            for _ in range(NPRE):
                nc.tensor.matmul(
                    wps[:, :],
                    warm_sb[:, 0:P],
                    warm_sb[:, P:],
                    start=True,
                    stop=True,
                    skip_group_check=True,
                )

    with tile.TileContext(nc) as tc:
        with (
            tc.tile_pool(name="stat", bufs=1) as stat,
            tc.tile_pool(name="psum", bufs=8, space="PSUM") as pp,
            tc.tile_pool(name="evict", bufs=4) as ep,
        ):
            # Input loads in k order so the k-outer matmul loop can start as
            # soon as the first k-tile pair lands; shift is only needed at
            # eviction time so it loads last.  DMA triggers cost ~600ns each
            # on the issuing engine, so round-robin them over three engines.
            trig = [nc.sync, nc.scalar, nc.gpsimd]
            ti = 0

            def dma(out, in_):
                nonlocal ti
                trig[ti % 3].dma_start(out=out, in_=in_)
                ti += 1

            if two_level:
                xw_t = []
                for k in range(KT):
                    ks = slice(k * P, (k + 1) * P)
                    t = stat.tile([P, width], dt_in, tag=f"xw{k}", name=f"xw{k}")
                    dma(t[:], xw_d[ks, :])
                    xw_t.append(t)
                xh_t = [t[:, 0:BS] for t in xw_t]
                wh_t = [t[:, BS : BS + OUT] for t in xw_t]
                xl_t = [t[:, BS + OUT : 2 * BS + OUT] for t in xw_t]
                wl_t = [t[:, 2 * BS + OUT :] for t in xw_t]
            else:
                # packed column order is [x m0..3 | W | x m4..7]: the first
                # BS//2 + OUT columns are all round 1 needs, so they load
                # first; the trailing x half is deferred until after them
                # (round 2 only starts reading it ~15us later).
                HB = BS // 2
                xwA_t, xB_t = [], []
                # k0 loads in two pieces so the n0-half groups can start
                # before W's n1 columns arrive.
                k0a = stat.tile([P, HB + NFREE], dt_in, tag="xwA0a", name="xwA0a")
                dma(k0a[:], xw_d[0:P, 0 : HB + NFREE])
                k0b = stat.tile([P, OUT - NFREE], dt_in, tag="xwA0b", name="xwA0b")
                dma(k0b[:], xw_d[0:P, HB + NFREE : HB + OUT])
                for k in range(1, KT):
                    ks = slice(k * P, (k + 1) * P)
                    t = stat.tile([P, HB + OUT], dt_in, tag=f"xwA{k}", name=f"xwA{k}")
                    dma(t[:], xw_d[ks, 0 : HB + OUT])
                    xwA_t.append(t)
                xwA_t.insert(0, None)
                for k in range(KT):
                    ks = slice(k * P, (k + 1) * P)
                    t = stat.tile([P, HB], dt_in, tag=f"xB{k}", name=f"xB{k}")
                    dma(t[:], xw_d[ks, HB + OUT :])
                    xB_t.append(t)
                xh_t = None  # x and w slices resolved via helpers below

                def w_slice(k, n):
                    if k == 0:
                        if n == 0:
                            return k0a[:, HB : HB + NFREE]
                        return k0b[:, (n - 1) * NFREE : n * NFREE]
                    return xwA_t[k][:, HB + n * NFREE : HB + (n + 1) * NFREE]
            if with_shift:
                shift_t = stat.tile([P, OUT], f32, tag="shift")
                dma(shift_t[:], shift_d[:])

            if two_level:
                passes = [(xh_t, wh_t), (xh_t, wl_t), (xl_t, wh_t)]

                def x_slice(xs, k, m):
                    return xs[k][:, m * P : (m + 1) * P]

                def w_slice(k, n):
                    raise NotImplementedError
            else:
                passes = [(None, None)]

                def x_slice(xs, k, m):
                    if m < MT // 2:
                        src_t = k0a if k == 0 else xwA_t[k]
                        return src_t[:, m * P : (m + 1) * P]
                    return xB_t[k][:, (m - MT // 2) * P : (m - MT // 2 + 1) * P]
            n_pass = len(passes)

            def evict(ps, m, n, gi, split=1):
                ms = slice(m * P, (m + 1) * P)
                ot = ep.tile([P, NFREE], f32, tag="ot", name=f"ot{gi}")
                h = NFREE // split
                for s in range(split):
                    fs = slice(s * h, (s + 1) * h)
                    ns = slice(n * NFREE + s * h, n * NFREE + (s + 1) * h)
                    if with_shift:
                        nc.vector.tensor_add(ot[:, fs], ps[:, fs], shift_t[:, ns])
                    else:
                        nc.vector.tensor_copy(ot[:, fs], ps[:, fs])
                    dma(out_d[ms, ns], ot[:, fs])

            # 16 output groups.  Round 1 (7 groups) runs contraction-outer so
            # the PE starts as soon as k-tile 0 lands instead of waiting for
            # the whole operand load; its evictions overlap round 2.  Round 2
            # (9 groups, all operands resident by then) runs contraction-inner
            # so groups finish staggered and the eviction tail after the last
            # matmul is a single tile, not a pile of eight.
            groups = [(m, n) for n in range(NT) for m in range(MT // 2)] + [
                (m, n) for m in range(MT // 2, MT) for n in range(NT)
            ]
            R1 = 8
            ps1 = [
                pp.tile([P, NFREE], f32, tag="ps", name=f"ps{gi}")
                for gi in range(R1)
            ]
            # PE warm-up: the HAM clock gate holds the PE at 1.2 GHz until it
            # has been busy ~3.4us.  The operand DMA takes ~4us to land, so a
            # stream of throwaway matmuls during the load makes the real ones
            # run at full clock from the start.  They accumulate into ps1[0],
            # which the first real matmul resets via start=True.
            NWARM = int(os.environ.get("BASS_DENSE_WARM", "32"))
            if NWARM:
                wa = stat.tile([P, P], dt_in, tag="warm_a")
                wb = stat.tile([P, P], dt_in, tag="warm_b")
                nc.vector.memset(wa[:], 0.0)
                nc.vector.memset(wb[:], 0.0)
            for i in range(NWARM):
                nc.tensor.matmul(
                    ps1[0][:, :P],
                    wa[:],
                    wb[:],
                    start=(i == 0),
                    stop=(i == NWARM - 1),
                )
            for k in range(KT):
                for gi, (m, n) in enumerate(groups[:R1]):
                    ms = slice(m * P, (m + 1) * P)
                    ns = slice(n * NFREE, (n + 1) * NFREE)
                    for pi, (xs, ws) in enumerate(passes):
                        rhs = ws[k][:, ns] if two_level else w_slice(k, n)
                        nc.tensor.matmul(
                            ps1[gi][:],
                            x_slice(xs, k, m),
                            rhs,
                            start=(k == 0 and pi == 0),
                            stop=(k == KT - 1 and pi == n_pass - 1),
                        )
            for gi, (m, n) in enumerate(groups[:R1]):
                evict(ps1[gi], m, n, gi)

            for gi, (m, n) in enumerate(groups[R1:], start=R1):
                ms = slice(m * P, (m + 1) * P)
                ns = slice(n * NFREE, (n + 1) * NFREE)
                last = gi == len(groups) - 1
                if last and not two_level:
                    # The very last group runs as two independent half-width
                    # accumulations: the first half stops eight matmuls early,
                    # so its eviction and output DMA overlap the second
                    # half's compute instead of trailing the final matmul.
                    ps = pp.tile([P, NFREE], f32, tag="ps", name=f"ps{gi}")
                    H = NFREE // 2
                    for s in range(2):
                        for idx, k in enumerate(range(KT)):
                            rhs = w_slice(k, n)[:, s * H : (s + 1) * H]
                            nc.tensor.matmul(
                                ps[:, s * H : (s + 1) * H],
                                x_slice(None, k, m),
                                rhs,
                                start=(idx == 0),
                                stop=(idx == KT - 1),
                            )
                        ot = ep.tile([P, H], f32, tag=f"otL{s}", name=f"otL{s}")
                        nsh = slice(n * NFREE + s * H, n * NFREE + (s + 1) * H)
                        if with_shift:
                            nc.vector.tensor_add(
                                ot[:], ps[:, s * H : (s + 1) * H], shift_t[:, nsh]
                            )
                        else:
                            nc.vector.tensor_copy(ot[:], ps[:, s * H : (s + 1) * H])
                        dma(out_d[ms, nsh], ot[:])
                    continue
                ps = pp.tile([P, NFREE], f32, tag="ps", name=f"ps{gi}")
                idx = 0
                for k in range(KT):
                    for xs, ws in passes:
                        rhs = ws[k][:, ns] if two_level else w_slice(k, n)
                        nc.tensor.matmul(
                            ps[:],
                            x_slice(xs, k, m),
                            rhs,
                            start=(idx == 0),
                            stop=(idx == KT * n_pass - 1),
                        )
                        idx += 1
                evict(ps, m, n, gi, split=(2 if gi >= len(groups) - 2 else 1))

    nc.compile()
    return nc


def _get_nc(mode, with_shift=True):
    key = (mode, with_shift)
    if key not in _cache:
        _cache[key] = _build(mode, with_shift)
    return _cache[key]


def kernel(inputs, W, b_vec, Aux, state_vec, depth=None, _trace=False):
    from concourse.bass_utils import run_bass_kernel_spmd

    inputs = np.asarray(inputs, dtype=np.float32)
    W = np.asarray(W, dtype=np.float32)
    b_vec = np.asarray(b_vec, dtype=np.float32)
    Aux = np.asarray(Aux, dtype=np.float32)
    state_vec = np.asarray(state_vec, dtype=np.float32)

    mode = MODE

    lo_dt = {"bf16": ml_dtypes.bfloat16, "fp16": np.float16}.get(mode[:4])

    wt = np.ascontiguousarray(W.T)  # [IN, OUT]
    shift = (b_vec + Aux[:, 0] * state_vec).astype(np.float32)
    with_shift = bool(np.any(shift))
    nc = _get_nc(mode, with_shift)

    if lo_dt is not None:
        wh = wt.astype(lo_dt)
        if mode.endswith("x3"):
            wl = (wt - wh.astype(np.float32)).astype(lo_dt)
    else:
        wh = wt.astype(np.float32)

    shift_rep = (
        np.ascontiguousarray(np.broadcast_to(shift, (P, OUT))) if with_shift else None
    )
    in_maps = []
    for c in range(N_CORES):
        xt = np.ascontiguousarray(inputs[c * BS : (c + 1) * BS].T)  # [IN, BS]
        if lo_dt is not None:
            xh = xt.astype(lo_dt)
            if mode.endswith("x3"):
                xl = (xt - xh.astype(np.float32)).astype(lo_dt)
                xw = np.concatenate([xh, wh, xl, wl], axis=1)
            else:
                xw = np.concatenate([xh[:, : BS // 2], wh, xh[:, BS // 2 :]], axis=1)
        else:
            xw = np.concatenate([xt[:, : BS // 2], wh, xt[:, BS // 2 :]], axis=1)
        m = {"xw": np.ascontiguousarray(xw)}
        if with_shift:
            m["shift"] = shift_rep
        in_maps.append(m)

    res = run_bass_kernel_spmd(nc, in_maps, list(range(N_CORES)), trace=_trace)
    out = np.concatenate([res.results[c]["out"] for c in range(N_CORES)], axis=0)

    # new_state: by linearity mean(pre, 0) == (mean_b inputs) @ W.T + b_vec.
    col_mean = inputs.sum(axis=0, dtype=np.float64) / B  # [IN]
    mean_pre = col_mean @ W.T.astype(np.float64) + b_vec
    new_state = (Aux[:, 1].astype(np.float64) * state_vec + mean_pre).astype(
        np.float32
    )

    if _trace:
        return (out, new_state), res
    return out, new_state


# revision 34
# speedup vs baseline: 2.2970x; 2.2970x over previous
"""Trainium2 Bass kernel for the recurrent Dense module.

Math (see reference):
    pre       = inputs @ W.T + b_vec            # [B, OUT]
    out       = pre + Aux[:,0] * state_vec      # [B, OUT]
    new_state = Aux[:,1] * state_vec + mean(pre, axis=0)   # [OUT]

Distribution: the batch (8192) is sharded over the 8 NeuronCores (1024 rows
each); W / b_vec / Aux / state_vec are replicated.  Each core computes its
[1024, OUT] slice of `out`; the host concatenates the slices.  `new_state`
depends on the batch only through mean(pre, 0) which by linearity equals
(mean_b inputs) @ W.T + b_vec, a [IN]x[IN,OUT] matvec the host computes
directly in float64.

On-chip layout per core: the matmul contracts over IN, so both operands are
staged with IN on the SBUF partition axis: lhsT = x_c.T (IN x 1024, the
stationary operand) and rhs = W.T (IN x OUT, the moving operand); both
transposes are done host-side during sharding.  PSUM tiles are
[128 batch, 512 out] fp32, accumulated over 8 k-tiles.  The affine shift
b_vec + Aux[:,0]*state_vec is folded in by the PSUM->SBUF eviction add.

Precision mode (BASS_DENSE_MODE env var, default fp16), measured on HW:
  fp16    1 matmul pass, operands rounded to fp16       (2.5e-4 rel err, ~47us)
  bf16    1 matmul pass, operands rounded to bf16       (2.0e-3 rel err, ~47us)
  bf16x3  3 passes hi/hi hi/lo lo/hi of a bf16 split    (3.8e-6 rel err, ~97us)
  fp32    native fp32 matmuls (4 cycles/row)            (bit-exact,     ~130us)
"""

import os

import ml_dtypes
import numpy as np

B, IN, OUT = 8192, 1024, 1024
N_CORES = 8
BS = B // N_CORES  # batch rows per core
P = 128  # SBUF partitions
NFREE = 512  # psum tile free dim (one bank of fp32)
KT = IN // P  # contraction tiles
MT = BS // P  # batch tiles per core
NT = OUT // NFREE  # out-feature tiles

MODE = os.environ.get("BASS_DENSE_MODE", "fp16")

_cache: dict = {}


def _build(mode, with_shift=True):
    import concourse.bacc as bacc
    import concourse.mybir as mybir
    import concourse.tile as tile

    nc = bacc.Bacc(enable_partition_id=False)
    f32 = mybir.dt.float32
    if mode in ("bf16", "bf16x3"):
        dt_in = mybir.dt.bfloat16
    elif mode in ("fp16", "fp16x3"):
        dt_in = mybir.dt.float16
    elif mode == "fp32":
        dt_in = f32
    else:
        raise ValueError(mode)
    two_level = mode in ("bf16x3", "fp16x3")

    # x and W (and their low halves in x3 modes) are packed side by side in
    # one [IN, width] tensor so each k-tile is a single DMA.
    width = (BS + OUT) * (2 if two_level else 1)
    xw_d = nc.dram_tensor("xw", [IN, width], dt_in, kind="ExternalInput")
    if with_shift:
        shift_d = nc.dram_tensor("shift", [P, OUT], f32, kind="ExternalInput")
    out_d = nc.dram_tensor("out", [BS, OUT], f32, kind="ExternalOutput")

    # PE warm-up, emitted in the main block so it runs during the framework
    # preamble, before the Tile block is even entered.  The HAM clock gate
    # needs ~3.4us of sustained PE activity before it lifts the PE from 1.2
    # to 2.4 GHz; these N=1 matmuls on the framework's const tiles provide it
    # so the real matmuls run at full clock from the first one.  The scratch
    # PSUM bank is freed again before the Tile pools allocate, and every real
    # accumulation group opens with start=True, which discards whatever the
    # warm-up wrote.
    NPRE = int(os.environ.get("BASS_DENSE_PREWARM", "0"))
    if NPRE:
        # Full-width (N=512) matmuls: narrow ones don't register enough PE
        # activity to trip the gate.  The scratch SBUF operand is read
        # uninitialized; its product is discarded, so the values are
        # irrelevant.
        warm_sb = nc.alloc_sbuf_tensor("warm_sb", [P, P + NFREE], mybir.dt.bfloat16)
        with nc.psum_tensor("warm_ps", [P, NFREE], f32) as wps:
            for _ in range(NPRE):
                nc.tensor.matmul(
                    wps[:, :],
                    warm_sb[:, 0:P],
                    warm_sb[:, P:],
                    start=True,
                    stop=True,
                    skip_group_check=True,
                )

    with tile.TileContext(nc) as tc:
        with (
            tc.tile_pool(name="stat", bufs=1) as stat,
            tc.tile_pool(name="psum", bufs=8, space="PSUM") as pp,
            tc.tile_pool(name="evict", bufs=4) as ep,
        ):
            # Input loads in k order so the k-outer matmul loop can start as
            # soon as the first k-tile pair lands; shift is only needed at
            # eviction time so it loads last.  DMA triggers cost ~600ns each
            # on the issuing engine, so round-robin them over three engines.
            trig = [nc.sync, nc.scalar, nc.gpsimd]
            ti = 0

            def dma(out, in_):
                nonlocal ti
                trig[ti % 3].dma_start(out=out, in_=in_)
                ti += 1

            if two_level:
                xw_t = []
                for k in range(KT):
                    ks = slice(k * P, (k + 1) * P)
                    t = stat.tile([P, width], dt_in, tag=f"xw{k}", name=f"xw{k}")
                    dma(t[:], xw_d[ks, :])
                    xw_t.append(t)
                xh_t = [t[:, 0:BS] for t in xw_t]
                wh_t = [t[:, BS : BS + OUT] for t in xw_t]
                xl_t = [t[:, BS + OUT : 2 * BS + OUT] for t in xw_t]
                wl_t = [t[:, 2 * BS + OUT :] for t in xw_t]
            else:
                # packed column order is [x m0..3 | W | x m4..7]: the first
                # BS//2 + OUT columns are all round 1 needs, so they load
                # first; the trailing x half is deferred until after them
                # (round 2 only starts reading it ~15us later).
                HB = BS // 2
                xwA_t, xB_t = [], []
                # The first KS k-tiles load in two pieces each so the n0-half
                # groups can start before W's n1 columns arrive -- finer
                # arrival gating while the DMA queues are still ramping up.
                KS = 1
                kA, kB = [], []
                for k in range(KS):
                    ks = slice(k * P, (k + 1) * P)
                    ta = stat.tile([P, HB + NFREE], dt_in, tag=f"xwA{k}a", name=f"xwA{k}a")
                    dma(ta[:], xw_d[ks, 0 : HB + NFREE])
                    kA.append(ta)
                    tb = stat.tile([P, OUT - NFREE], dt_in, tag=f"xwA{k}b", name=f"xwA{k}b")
                    dma(tb[:], xw_d[ks, HB + NFREE : HB + OUT])
                    kB.append(tb)
                for k in range(KS, KT):
                    ks = slice(k * P, (k + 1) * P)
                    t = stat.tile([P, HB + OUT], dt_in, tag=f"xwA{k}", name=f"xwA{k}")
                    dma(t[:], xw_d[ks, 0 : HB + OUT])
                    xwA_t.append(t)
                xwA_t = [None] * KS + xwA_t
                for k in range(KT):
                    ks = slice(k * P, (k + 1) * P)
                    t = stat.tile([P, HB], dt_in, tag=f"xB{k}", name=f"xB{k}")
                    dma(t[:], xw_d[ks, HB + OUT :])
                    xB_t.append(t)
                xh_t = None  # x and w slices resolved via helpers below

                def w_slice(k, n):
                    if k < KS:
                        if n == 0:
                            return kA[k][:, HB : HB + NFREE]
                        return kB[k][:, (n - 1) * NFREE : n * NFREE]
                    return xwA_t[k][:, HB + n * NFREE : HB + (n + 1) * NFREE]
            if with_shift:
                shift_t = stat.tile([P, OUT], f32, tag="shift")
                dma(shift_t[:], shift_d[:])

            if two_level:
                passes = [(xh_t, wh_t), (xh_t, wl_t), (xl_t, wh_t)]

                def x_slice(xs, k, m):
                    return xs[k][:, m * P : (m + 1) * P]

                def w_slice(k, n):
                    raise NotImplementedError
            else:
                passes = [(None, None)]

                def x_slice(xs, k, m):
                    if m < MT // 2:
                        src_t = kA[k] if k < KS else xwA_t[k]
                        return src_t[:, m * P : (m + 1) * P]
                    return xB_t[k][:, (m - MT // 2) * P : (m - MT // 2 + 1) * P]
            n_pass = len(passes)

            def evict(ps, m, n, gi, split=1):
                ms = slice(m * P, (m + 1) * P)
                ot = ep.tile([P, NFREE], f32, tag="ot", name=f"ot{gi}")
                h = NFREE // split
                for s in range(split):
                    fs = slice(s * h, (s + 1) * h)
                    ns = slice(n * NFREE + s * h, n * NFREE + (s + 1) * h)
                    if with_shift:
                        nc.vector.tensor_add(ot[:, fs], ps[:, fs], shift_t[:, ns])
                    else:
                        nc.vector.tensor_copy(ot[:, fs], ps[:, fs])
                    dma(out_d[ms, ns], ot[:, fs])

            # 16 output groups.  Round 1 (7 groups) runs contraction-outer so
            # the PE starts as soon as k-tile 0 lands instead of waiting for
            # the whole operand load; its evictions overlap round 2.  Round 2
            # (9 groups, all operands resident by then) runs contraction-inner
            # so groups finish staggered and the eviction tail after the last
            # matmul is a single tile, not a pile of eight.
            groups = [(m, n) for n in range(NT) for m in range(MT // 2)] + [
                (m, n) for m in range(MT // 2, MT) for n in range(NT)
            ]
            R1 = 8
            ps1 = [
                pp.tile([P, NFREE], f32, tag="ps", name=f"ps{gi}")
                for gi in range(R1)
            ]
            # PE warm-up: the HAM clock gate holds the PE at 1.2 GHz until it
            # has been busy ~3.4us.  The operand DMA takes ~4us to land, so a
            # stream of throwaway matmuls during the load makes the real ones
            # run at full clock from the start.  They accumulate into ps1[0],
            # which the first real matmul resets via start=True.
            NWARM = int(os.environ.get("BASS_DENSE_WARM", "32"))
            if NWARM:
                wa = stat.tile([P, P], dt_in, tag="warm_a")
                wb = stat.tile([P, P], dt_in, tag="warm_b")
                nc.vector.memset(wa[:], 0.0)
                nc.vector.memset(wb[:], 0.0)
            for i in range(NWARM):
                nc.tensor.matmul(
                    ps1[0][:, :P],
                    wa[:],
                    wb[:],
                    start=(i == 0),
                    stop=(i == NWARM - 1),
                )
            for k in range(KT):
                for gi, (m, n) in enumerate(groups[:R1]):
                    ms = slice(m * P, (m + 1) * P)
                    ns = slice(n * NFREE, (n + 1) * NFREE)
                    for pi, (xs, ws) in enumerate(passes):
                        rhs = ws[k][:, ns] if two_level else w_slice(k, n)
                        nc.tensor.matmul(
                            ps1[gi][:],
                            x_slice(xs, k, m),
                            rhs,
                            start=(k == 0 and pi == 0),
                            stop=(k == KT - 1 and pi == n_pass - 1),
                        )
            for gi, (m, n) in enumerate(groups[:R1]):
                evict(ps1[gi], m, n, gi)

            for gi, (m, n) in enumerate(groups[R1:], start=R1):
                ms = slice(m * P, (m + 1) * P)
                ns = slice(n * NFREE, (n + 1) * NFREE)
                last = gi == len(groups) - 1
                if last and not two_level:
                    # The very last group runs as two independent half-width
                    # accumulations: the first half stops eight matmuls early,
                    # so its eviction and output DMA overlap the second
                    # half's compute instead of trailing the final matmul.
                    ps = pp.tile([P, NFREE], f32, tag="ps", name=f"ps{gi}")
                    H = NFREE // 2
                    for s in range(2):
                        for idx, k in enumerate(range(KT)):
                            rhs = w_slice(k, n)[:, s * H : (s + 1) * H]
                            nc.tensor.matmul(
                                ps[:, s * H : (s + 1) * H],
                                x_slice(None, k, m),
                                rhs,
                                start=(idx == 0),
                                stop=(idx == KT - 1),
                            )
                        ot = ep.tile([P, H], f32, tag=f"otL{s}", name=f"otL{s}")
                        nsh = slice(n * NFREE + s * H, n * NFREE + (s + 1) * H)
                        if with_shift:
                            nc.vector.tensor_add(
                                ot[:], ps[:, s * H : (s + 1) * H], shift_t[:, nsh]
                            )
                        else:
                            nc.vector.tensor_copy(ot[:], ps[:, s * H : (s + 1) * H])
                        dma(out_d[ms, nsh], ot[:])
                    continue
                ps = pp.tile([P, NFREE], f32, tag="ps", name=f"ps{gi}")
                idx = 0
                for k in range(KT):
                    for xs, ws in passes:
                        rhs = ws[k][:, ns] if two_level else w_slice(k, n)
                        nc.tensor.matmul(
                            ps[:],
                            x_slice(xs, k, m),
                            rhs,
                            start=(idx == 0),
                            stop=(idx == KT * n_pass - 1),
                        )
                        idx += 1
                evict(ps, m, n, gi, split=(2 if gi >= len(groups) - 2 else 1))

    nc.compile()
    return nc


def _get_nc(mode, with_shift=True):
    key = (mode, with_shift)
    if key not in _cache:
        _cache[key] = _build(mode, with_shift)
    return _cache[key]


def kernel(inputs, W, b_vec, Aux, state_vec, depth=None, _trace=False):
    from concourse.bass_utils import run_bass_kernel_spmd

    inputs = np.asarray(inputs, dtype=np.float32)
    W = np.asarray(W, dtype=np.float32)
    b_vec = np.asarray(b_vec, dtype=np.float32)
    Aux = np.asarray(Aux, dtype=np.float32)
    state_vec = np.asarray(state_vec, dtype=np.float32)

    mode = MODE

    lo_dt = {"bf16": ml_dtypes.bfloat16, "fp16": np.float16}.get(mode[:4])

    wt = np.ascontiguousarray(W.T)  # [IN, OUT]
    shift = (b_vec + Aux[:, 0] * state_vec).astype(np.float32)
    with_shift = bool(np.any(shift))
    nc = _get_nc(mode, with_shift)

    if lo_dt is not None:
        wh = wt.astype(lo_dt)
        if mode.endswith("x3"):
            wl = (wt - wh.astype(np.float32)).astype(lo_dt)
    else:
        wh = wt.astype(np.float32)

    shift_rep = (
        np.ascontiguousarray(np.broadcast_to(shift, (P, OUT))) if with_shift else None
    )
    in_maps = []
    for c in range(N_CORES):
        xt = np.ascontiguousarray(inputs[c * BS : (c + 1) * BS].T)  # [IN, BS]
        if lo_dt is not None:
            xh = xt.astype(lo_dt)
            if mode.endswith("x3"):
                xl = (xt - xh.astype(np.float32)).astype(lo_dt)
                xw = np.concatenate([xh, wh, xl, wl], axis=1)
            else:
                xw = np.concatenate([xh[:, : BS // 2], wh, xh[:, BS // 2 :]], axis=1)
        else:
            xw = np.concatenate([xt[:, : BS // 2], wh, xt[:, BS // 2 :]], axis=1)
        m = {"xw": np.ascontiguousarray(xw)}
        if with_shift:
            m["shift"] = shift_rep
        in_maps.append(m)

    res = run_bass_kernel_spmd(nc, in_maps, list(range(N_CORES)), trace=_trace)
    out = np.concatenate([res.results[c]["out"] for c in range(N_CORES)], axis=0)

    # new_state: by linearity mean(pre, 0) == (mean_b inputs) @ W.T + b_vec.
    col_mean = inputs.sum(axis=0, dtype=np.float64) / B  # [IN]
    mean_pre = col_mean @ W.T.astype(np.float64) + b_vec
    new_state = (Aux[:, 1].astype(np.float64) * state_vec + mean_pre).astype(
        np.float32
    )

    if _trace:
        return (out, new_state), res
    return out, new_state
